# revision 2
# baseline (speedup 1.0000x reference)
"""Distributed Trainium2 Bass kernel for a single-head causal attention layer.

Problem: x[8, 2048, 1024] -> per batch element: q/k/v = x @ W* + b*;
out = causal_softmax(q k^T / sqrt(64)) @ v   -> [8, 2048, 64]

Sharding: pure data parallel over the batch dim - core i computes batch
element i. No collectives.

V3 pipeline (bf16 matmuls; fp8 was measured at 2.4e-2..7.7e-2 rel err,
over the 2e-2 budget, so everything stays >= bf16):
  1. One DMA per 512-row group loads x [128, 4, E] fp32; the idle Pool
     engine converts to bf16 (per-column-chunk, pipelined); PE transposes
     128x128 bf16 tiles into [128, 2, 512] PSUM pairs (1 bank); one DVE
     copy per pair moves them to SBUF.
  2. Projections with stacked stationary bf16 weights [Wq|Wk] and [Wv|Wv]
     accumulate in PSUM; DVE bias-copies produce qkT bf16 (partitions
     0:64 = qT, 64:128 = kT) and vT bf16.
  3. qT is replicated to partitions 64:128 (qdup) and kT to 0:64 (kdup)
     by two SBUF DMAs per group, so the two scores matmuls of a j-tile
     pair occupy disjoint PE row groups and run concurrently.
  4. vT j-tiles are PE-transposed to natural layout and copied (ScalarE)
     into vaug [128, jt, 96]: 64 v cols + 32 ones cols (rowsum-via-matmul
     for the softmax denominator).
  5. Scores computed TRANSPOSED per j-tile over its live i-window;
     additive -1e9 triangular mask on diagonal tiles only; j-tiles above
     the diagonal skipped. No max-subtraction (|logits| <~ 6).
  6. p^T = exp(0.125 * sT) on ScalarE straight from PSUM, bf16.
  7. outT[h', i] += vaug[jt]^T pT[jt] accumulated over j-tiles in PSUM;
     rows 64:96 accumulate the denominator.
  8. Transpose [96,128] chunks back, multiply by the reciprocal
     denominator, one output DMA per 512-row block.

Scheduling: attention is emitted as per-pair generators pumped between
projection chunks, so score->exp->AV latency chains hide under dense PE
work; two sub-iterations are emitted per For_i trip for real double
buffering; the x-load ring (SP) carries only x loads.

Engines: PE matmuls/transposes; DVE PSUM->SBUF copies, bias-adds, masks,
reciprocal; ScalarE exp + vaug/final copies; Pool x->bf16 converts +
ones memset; SP ring x loads + dup DMAs; scalar ring consts and outputs.
"""

import numpy as np

# ---------------------------------------------------------------------------
# Workarounds for the installed walrus build, which rejects any instruction
# carrying more than one sync-wait command.
# ---------------------------------------------------------------------------
import bass_rust
import concourse.bass as bass
import concourse.mybir as mybir
import concourse.tile as tile
from concourse.vector_clock import ScopedClock

_split_counter = [0]


def _patched_drain_and_barrier(self, tick_clock, wait_clock):
    nc = self.nc
    collector = nc.sync.nop(hint="drain_wait_split", nofuse=True)
    wait_clock.add_sem_waits(
        collector.ins, ScopedClock({None: tick_clock.global_clock})
    )
    si = collector.ins.sync_info
    if si is not None and si.on_wait and len(si.on_wait) > 1:
        extra = list(si.on_wait[1:])
        del si.on_wait[1:]
        for w in extra:
            nop = nc.sync.nop(hint="drain_wait_split", nofuse=True)
            nop.ins.sync_info = mybir.SyncInfo(on_wait=[w], on_update=[])
    nc.sync.drain()
    nc.all_engine_barrier()
    assert self.sems is not None
    popped = nc._tile_sem_poison_stack.pop()
    assert popped is self._sem_poison
    nc.clear_and_free_semaphores(list(self.sems.allocated().values()))
    nc.all_engine_barrier()


tile.TileContext._drain_and_barrier = _patched_drain_and_barrier


def split_multi_waits(nc, max_waits: int = 1) -> int:
    """Hoist extra sync-waits onto same-engine nops placed just before the
    instruction. Waits are preconditions executed by the engine sequencer in
    program order, so this is behavior-preserving."""
    n_inserted = 0
    for func in nc.m.functions:
        for bb in func.blocks:
            if not any(
                i.sync_info is not None and len(i.sync_info.on_wait) > max_waits
                for i in bb.instructions
            ):
                continue
            new_insts = []
            for inst in bb.instructions:
                si = inst.sync_info
                if si is not None and len(si.on_wait) > max_waits:
                    keep_from = len(si.on_wait) - max_waits
                    extra = list(si.on_wait[:keep_from])
                    keep = list(si.on_wait[keep_from:])
                    for w in extra:
                        _split_counter[0] += 1
                        nop = bass_rust.InstNoOp(
                            name=f"I-wsplit-{_split_counter[0]}",
                            engine=inst.engine,
                        )
                        nop.sync_info = mybir.SyncInfo(on_wait=[w], on_update=[])
                        nc.register_instruction(nop, overwrite=True)
                        new_insts.append(nop)
                        n_inserted += 1
                    del si.on_wait[:]
                    si.on_wait.extend(keep)
                new_insts.append(inst)
            bb.instructions[:] = new_insts
    return n_inserted


# ---------------------------------------------------------------------------
# Problem constants (hardcoded per the harness contract).
# ---------------------------------------------------------------------------
B, T, E, H = 8, 2048, 1024, 64
N_CORES = 8
P = 128                      # partitions / tile edge
NT = T // 512                # 4 N-chunks of 512 over T
ET = E // P                  # 8 contraction tiles over E
JT = T // P                  # 16 j-tiles
VA = 96                      # v_aug width: 64 v cols + 32 ones cols
SCALE = 1.0 / np.sqrt(H)     # 0.125
MASK_NEG = -1.0e9

PAIRED = True  # two concurrent scores matmuls in disjoint PE row groups

F32 = mybir.dt.float32
F32R = mybir.dt.float32r
BF16 = mybir.dt.bfloat16
EXP = mybir.ActivationFunctionType.Exp
CPY = mybir.ActivationFunctionType.Copy
ADD = mybir.AluOpType.add


def build_bass(n_iters: int = 1, t_size: int = T, sim_unroll: bool = False):
    nt = t_size // 512
    jt_n = t_size // P
    nc = bass.Bass()

    xp = nc.declare_dram_parameter("x", [t_size, E], F32R, isOutput=False)
    wqkp = nc.declare_dram_parameter("wqk", [E, P], BF16, isOutput=False)
    wvvp = nc.declare_dram_parameter("wvv", [E, P], BF16, isOutput=False)
    bqp = nc.declare_dram_parameter("bq", [H], F32, isOutput=False)
    bkp = nc.declare_dram_parameter("bk", [H], F32, isOutput=False)
    bvp = nc.declare_dram_parameter("bv", [H], F32, isOutput=False)
    idbp = nc.declare_dram_parameter("identb", [P, P], BF16, isOutput=False)
    idrp = nc.declare_dram_parameter("identr", [P, P], F32R, isOutput=False)
    dmp = nc.declare_dram_parameter("dmask", [P, P], F32, isOutput=False)
    outp = nc.declare_dram_parameter("out", [t_size, H], F32, isOutput=True)

    with tile.TileContext(nc) as tc:
        with (
            tc.tile_pool(name="consts", bufs=1) as consts,
            tc.tile_pool(name="xa", bufs=3) as xa_pool,
            tc.tile_pool(name="xb", bufs=3) as xb_pool,
            tc.tile_pool(name="big", bufs=2) as big,
            tc.tile_pool(name="xt", bufs=8) as xt_pool,
            tc.tile_pool(name="work", bufs=4) as work,
            tc.tile_pool(name="small", bufs=4) as small,
            tc.tile_pool(name="ps_mm", bufs=1, space="PSUM") as ps_mm,
            tc.tile_pool(name="ps_sc", bufs=3, space="PSUM") as ps_sc,
            tc.tile_pool(name="ps_out", bufs=2, space="PSUM") as ps_out,
            tc.tile_pool(name="ps_tr", bufs=2, space="PSUM") as ps_tr,
        ):
            # ---- constants / weights ----
            identb = consts.tile([P, P], BF16)
            nc.scalar.dma_start(out=identb, in_=idbp[:])
            identr = consts.tile([P, P], F32R)
            nc.scalar.dma_start(out=identr, in_=idrp[:])
            dmask = consts.tile([P, P], F32)
            nc.scalar.dma_start(out=dmask, in_=dmp[:])

            wqk = consts.tile([P, ET, P], BF16)   # [e-part, e-tile, (q|k)]
            nc.scalar.dma_start(
                out=wqk, in_=wqkp.rearrange("(et p) h -> p et h", p=P)
            )
            wv1 = consts.tile([P, ET, H], BF16)   # [e-part, e-tile, h]
            nc.scalar.dma_start(
                out=wv1, in_=wvvp.rearrange("(et p) h -> p et h", p=P)[:, :, 0:H]
            )
            bqk = consts.tile([P, 1], F32)
            nc.scalar.dma_start(out=bqk[0:H], in_=bqp.rearrange("(h one) -> h one", one=1))
            nc.scalar.dma_start(out=bqk[H:P], in_=bkp.rearrange("(h one) -> h one", one=1))
            bvrow = consts.tile([P, 4, H], F32)
            nc.scalar.dma_start(
                out=bvrow,
                in_=bvp.rearrange("(one h) -> one h", one=1).broadcast_to(
                    (P * 4, H)
                ).rearrange("(p c) h -> p c h", p=P),
            )

            from contextlib import nullcontext

            # Two alternating sub-iterations per For_i trip: tile() calls
            # inside a hardware loop bind static buffers, so ring rotation
            # only happens across *emitted* sub-bodies. Two emissions give
            # true double buffering and let the attention tail of sub-body A
            # drain under sub-body B's projections.
            if n_iters == 1:
                loop_ctx, n_unroll = nullcontext(), 1
            elif sim_unroll:
                loop_ctx, n_unroll = nullcontext(), n_iters
            else:
                assert n_iters % 2 == 0, "n_iters must be even for For_i"
                loop_ctx, n_unroll = tc.For_i(0, n_iters // 2, 1), 2

            with loop_ctx:
                # Per-sub-iteration rotating buffers (bufs=2).
                state = {}

                def alloc_iter():
                    # vaug [j-part, jt, 96]: data cols 0:64 rewritten per
                    # group; ones cols re-filled each pass (buffer rotates).
                    vaug = big.tile([P, jt_n, VA], BF16, tag="vaug")
                    nc.gpsimd.memset(vaug[:, :, H:VA], 1.0)
                    # qkT / qdup / kdup: [128, nt, 512] bf16. qkT holds qT on
                    # partitions 0:64 and kT on 64:128; qdup replicates qT at
                    # 64:128, kdup replicates kT at 0:64 (PE row-group
                    # pairing for concurrent scores matmuls).
                    qkT = big.tile([P, nt, 512], BF16, tag="qkT")
                    qdup = big.tile([P, nt, 512], BF16, tag="qdup")
                    kdup = big.tile([P, nt, 512], BF16, tag="kdup")
                    state["vaug"] = vaug
                    state["qkT"] = qkT
                    state["qdup"] = qdup
                    state["kdup"] = kdup

                def stage_a_load(g):
                    # one DMA loads the whole 512-row group [128, 4, E]
                    xa = xa_pool.tile([P, 4, E], F32R, tag="xa")
                    nc.sync.dma_start(
                        out=xa,
                        in_=xp[g * 512 : (g + 1) * 512, :].rearrange(
                            "(k p) e -> p k e", p=P
                        ),
                    )
                    return xa

                def xpose_chunk(xa, xb, et, xt):
                    # fp32r transposes straight from the x tile (1.5 c/row on
                    # PE, but no staging convert); the DVE PSUM->SBUF copy
                    # converts to bf16. One copy per chunk pair.
                    if et % 2 == 0:
                        state["psb"] = ps_tr.tile(
                            [P, 512], F32R, tag="ptr", name=f"psb_{et}"
                        )
                        state["psb2"] = ps_tr.tile(
                            [P, 512], F32R, tag="ptr", name=f"psb2_{et}"
                        )
                    psb = state["psb"] if et % 2 == 0 else state["psb2"]
                    for k in range(4):
                        nc.tensor.transpose(
                            psb[:, k * P : (k + 1) * P],
                            xa[:, k, et * P : (et + 1) * P],
                            identr,
                        )
                    xtt = xt_pool.tile([P, 512], BF16, tag="xt", name=f"xt_{et}")
                    nc.vector.tensor_copy(out=xtt, in_=psb)
                    xt.append(xtt)

                def proj_mms(dst, w, xt, pump):
                    for et in range(ET):
                        nc.tensor.matmul(
                            dst,
                            w[:, et, :],
                            xt[et],
                            start=(et == 0),
                            stop=(et == ET - 1),
                        )
                        if et % 4 == 3:
                            pump(1)

                def proj(g, xt, pump):
                    psa = ps_mm.tile([P, 512], F32, tag="mm")
                    proj_mms(psa, wqk, xt, pump)
                    qkT = state["qkT"]
                    nc.vector.tensor_scalar(
                        out=qkT[:, g, :], in0=psa, scalar1=bqk, scalar2=None, op0=ADD
                    )
                    # row-group replicas for paired scores matmuls
                    dup_eng = nc.scalar if g == nt - 1 else nc.sync
                    dup_eng.dma_start(
                        out=state["qdup"][H:P, g, :], in_=qkT[0:H, g, :]
                    )
                    dup_eng.dma_start(
                        out=state["kdup"][0:H, g, :], in_=qkT[H:P, g, :]
                    )

                    # v-pass in stationary-xT form: lhsT = xT e-chunk
                    # (stationary, t on the free dim), moving = Wv. Output is
                    # v NATURAL [t-part, tile, h] - no transposes needed, and
                    # the free-dim moving operand is only 64 wide so this
                    # pass costs half the qk pass. The bias is folded into
                    # the finalize (out += bv after normalization, since
                    # sum_j p_j (v_j + bv) / sum p = out + bv).
                    psv = ps_mm.tile([P, 4, H], F32, tag="mm")
                    for tt in range(4):
                        for et in range(ET):
                            nc.tensor.matmul(
                                psv[:, tt, :],
                                xt[et][:, tt * P : (tt + 1) * P],
                                wv1[:, et, :],
                                start=(et == 0),
                                stop=(et == ET - 1),
                            )
                        if tt % 2 == 1:
                            pump(1)
                    nc.vector.tensor_copy(
                        out=state["vaug"][:, 4 * g : 4 * g + 4, 0:H],
                        in_=psv,
                    )

                def vaug_group(g, vt):
                    pass

                def attn_gen(ib, st):
                    """Generator: one yield per j-tile pair, then the
                    finalize. Pumped between projection chunks so pair
                    latency chains overlap dense PE work. `st` snapshots the
                    sub-iteration's buffers at creation time."""
                    pso = ps_out.tile([VA, 512], F32, tag="out")
                    n_jt = 4 * ib + 4
                    qkT, qdup, kdup = st["qkT"], st["qdup"], st["kdup"]
                    vaug = st["vaug"]

                    def emit_scores_exp(jt, hi):
                        # hi: rows 64:128 (kT from qkT, qT from qdup);
                        # lo: rows 0:64 (qT from qkT, kT from kdup).
                        lo = max(0, jt * P - ib * 512)
                        n = 512 - lo
                        gj, jc = jt // 4, (jt * P) % 512
                        pss = ps_sc.tile([P, 512], F32, tag="sc")
                        if hi and PAIRED:
                            lhsT = qkT[H:P, gj, jc : jc + P]
                            rhs = qdup[H:P, ib, lo:512]
                        else:
                            lhsT = kdup[0:H, gj, jc : jc + P]
                            rhs = qkT[0:H, ib, lo:512]
                        nc.tensor.matmul(
                            pss[:, 0:n], lhsT, rhs, start=True, stop=True
                        )
                        if jt >= 4 * ib:  # diagonal tile: causal mask
                            nc.vector.tensor_add(
                                out=pss[:, 0:P], in0=pss[:, 0:P], in1=dmask
                            )
                        pt = work.tile([P, 512], BF16, tag="pt")
                        nc.scalar.activation(
                            out=pt[:, 0:n],
                            in_=pss[:, 0:n],
                            func=EXP,
                            scale=SCALE,
                        )
                        return pt, lo, n

                    def emit_av(jt, pt, lo, n):
                        nc.tensor.matmul(
                            pso[:, lo:512],
                            vaug[:, jt, :],
                            pt[:, 0:n],
                            start=(jt == 0),
                            stop=(jt == n_jt - 1),
                        )

                    # pair-skewed pipeline: the two scores of a row-group
                    # pair run ahead of the matching AV pair
                    pend = []
                    for jt0 in range(0, n_jt, 2):
                        pair = [
                            (jt0, emit_scores_exp(jt0, hi=False)),
                            (jt0 + 1, emit_scores_exp(jt0 + 1, hi=True)),
                        ]
                        for jt, args in pend:
                            emit_av(jt, *args)
                        pend = pair
                        yield
                    for jt, args in pend:
                        emit_av(jt, *args)

                    # finalize: transpose [96, 128] chunks back to [128, 96]
                    # (64 data cols + denominator cols), divide, store.
                    ot = work.tile([VA, 512], F32R, tag="oT")
                    nc.vector.tensor_copy(out=ot, in_=pso)
                    psf = ps_sc.tile([P, 4 * VA], F32R, tag="sc")
                    for c in range(4):
                        nc.tensor.transpose(
                            psf[:, c * VA : (c + 1) * VA],
                            ot[:, c * P : (c + 1) * P],
                            identr[0:VA, 0:VA],
                        )
                    yield
                    osb = small.tile([P, 4, H], F32, tag="osb")
                    for c in range(4):
                        rs = small.tile([P, 1], F32, tag="rs")
                        nc.vector.reciprocal(
                            rs, psf[:, c * VA + H : c * VA + H + 1]
                        )
                        nc.vector.tensor_scalar_mul(
                            out=osb[:, c, :],
                            in0=psf[:, c * VA : c * VA + H],
                            scalar1=rs,
                        )
                    nc.vector.tensor_tensor(
                        out=osb, in0=osb, in1=bvrow, op=ADD
                    )
                    nc.scalar.dma_start(
                        out=outp[ib * 512 : (ib + 1) * 512, :].rearrange(
                            "(c p) h -> p c h", p=P
                        ),
                        in_=osb,
                    )

                # Interleave attention pairs into the projection stream: a
                # FIFO of attn generators is pumped between transpose /
                # matmul chunks, so pair latency chains (score -> exp -> AV)
                # hide under dense PE work.
                from collections import deque

                gens = deque()

                def pump(k=1):
                    while k > 0 and gens:
                        try:
                            next(gens[0])
                            k -= 1
                        except StopIteration:
                            gens.popleft()

                for _ in range(n_unroll):
                    alloc_iter()
                    # prefetch x two groups ahead so the load ring never
                    # waits behind the dup DMAs of the current group
                    xas = [stage_a_load(g) for g in range(min(2, nt))]
                    for g in range(nt):
                        if g + 2 < nt:
                            xas.append(stage_a_load(g + 2))
                        xa = xas[g]
                        xb = None
                        xt = []
                        for et in range(ET):
                            xpose_chunk(xa, xb, et, xt)
                            if et % 2 == 1:
                                pump(1)
                        proj(g, xt, pump)
                        gens.append(attn_gen(g, dict(state)))
                        pump(2)
                # drain once per emitted body: only the last sub-iteration's
                # attention tail is not hidden under other work
                while gens:
                    pump(1)

    split_multi_waits(nc)
    return nc


# ---------------------------------------------------------------------------
# Host-side wrapper
# ---------------------------------------------------------------------------
def _consts_inputs(t_size=T):
    import ml_dtypes

    identb = np.eye(P, dtype=ml_dtypes.bfloat16)
    identr = np.eye(P, dtype=np.float32)
    # scores^T[j, i_local]: valid j <= i_local; mask j > i_local
    j = np.arange(P)[:, None]
    i = np.arange(P)[None, :]
    dmask = np.where(j <= i, 0.0, MASK_NEG).astype(np.float32)
    return {"identb": identb, "identr": identr, "dmask": dmask}


def _weights_inputs(Wq, Wk, Wv):
    import ml_dtypes

    wqk = np.concatenate(
        [np.asarray(Wq, np.float32), np.asarray(Wk, np.float32)], axis=1
    ).astype(ml_dtypes.bfloat16)
    wv = np.asarray(Wv, np.float32)
    wvv = np.concatenate([wv, wv], axis=1).astype(ml_dtypes.bfloat16)
    return {"wqk": wqk, "wvv": wvv}


def kernel(x, Wq, bq, Wk, bk, Wv, bv, _nc_cache={}):
    from concourse.bass_utils import run_bass_kernel_spmd

    if "nc" not in _nc_cache:
        _nc_cache["nc"] = build_bass(n_iters=1)
    nc = _nc_cache["nc"]

    x = np.asarray(x, dtype=np.float32)
    consts = _consts_inputs()
    weights = _weights_inputs(Wq, Wk, Wv)
    in_maps = []
    for c in range(N_CORES):
        m = {
            "x": np.ascontiguousarray(x[c]),
            "bq": np.asarray(bq, np.float32),
            "bk": np.asarray(bk, np.float32),
            "bv": np.asarray(bv, np.float32),
        }
        m.update(consts)
        m.update(weights)
        in_maps.append(m)

    res = run_bass_kernel_spmd(nc, in_maps, core_ids=list(range(N_CORES)))
    out = np.stack([res.results[c]["out"] for c in range(N_CORES)], axis=0)
    return out
